# revision 4
# baseline (speedup 1.0000x reference)
"""Causal multi-head attention block (B=2, S=2048, D=1024, H=16) on 8 TRN2 cores.

Sharding: core i handles batch b = i//4 and head group hg = i%4 (4 heads =
256 model dims). Each core computes its heads' attention and a partial
output projection; the host sums the 4 partials per batch and adds b_out.

Per-core device pipeline (bf16 matmuls, fp32 PSUM accumulation):
  1. QKV. Q^T,K^T land as [head_cols, tokens] (lhsT = W, rhs = x^T);
     V lands as [tokens, head_cols] (lhsT = x^T tiles, rhs = W_v) and is
     stored augmented with a ones column so the attention z-matmul also
     produces softmax row sums.
  2. Attention per head, flash-style in the S^T = K.Q^T orientation over
     the causal lower triangle only: S^T[k_tile, q_span] -> +tri mask on
     the diagonal block -> exp on ScalarE (scale=1/8, no max subtraction:
     logits are ~N(0,1)) -> P^T bf16 -> z^T[d+1, q] += V_aug^T @ P^T
     accumulated over k tiles in PSUM.
  3. Normalize: recip(rowsum) -> broadcast across partitions -> z * recip
     + b_v on VectorE -> bf16, DMA'd into the out-proj lhsT layout.
  4. Out-proj: y_partial[t, n] accumulated over the 256 local dims.
"""

import numpy as np
import ml_dtypes

import concourse.mybir as mybir
import concourse.tile as tile
from concourse import bacc
from concourse.bass_utils import run_bass_kernel_spmd

B = 2
S = 2048
DM = 1024
HD = 64
HLOC = 4                 # heads per core
CLOC = HLOC * HD         # local model dims (256)
MO = DM // 128           # 8 k-subtiles of the model dim
NKT = S // 128           # 16 key tiles
NEG = -1e30

f32 = mybir.dt.float32
bf16 = mybir.dt.bfloat16
EXP = mybir.ActivationFunctionType.Exp

_CACHE = {}


def build():
    nc = bacc.Bacc("TRN2", target_bir_lowering=False, debug=False)

    xT_d = nc.dram_tensor("xT", [DM, S], bf16, kind="ExternalInput")
    wq_d = nc.dram_tensor("wq", [DM, CLOC], bf16, kind="ExternalInput")
    wk_d = nc.dram_tensor("wk", [DM, CLOC], bf16, kind="ExternalInput")
    wv_d = nc.dram_tensor("wv", [DM, CLOC], bf16, kind="ExternalInput")
    wo_d = nc.dram_tensor("wo", [CLOC, DM], bf16, kind="ExternalInput")
    bq_d = nc.dram_tensor("bq", [CLOC], f32, kind="ExternalInput")
    bk_d = nc.dram_tensor("bk", [CLOC], f32, kind="ExternalInput")
    bv_d = nc.dram_tensor("bv", [64, HLOC], f32, kind="ExternalInput")
    tri_d = nc.dram_tensor("tri", [128, 128], f32, kind="ExternalInput")
    y_d = nc.dram_tensor("y", [S, DM], f32, kind="ExternalOutput")

    with tile.TileContext(nc) as tc:
        with (
            tc.tile_pool(name="consts", bufs=1) as consts,
            tc.tile_pool(name="acts", bufs=1) as apool,
            tc.tile_pool(name="pt", bufs=3) as ppool,
            tc.tile_pool(name="norm", bufs=4) as spool,
            tc.tile_pool(name="ycopy", bufs=4) as ypool,
            tc.tile_pool(name="ps_s", bufs=2, space="PSUM") as ps_s,
            tc.tile_pool(name="ps_z", bufs=2, space="PSUM") as ps_z,
            tc.tile_pool(name="ps_b", bufs=2, space="PSUM") as ps_b,
        ):
            # ---- constants / weights -------------------------------------
            tri = consts.tile([128, 128], f32)
            nc.sync.dma_start(tri[:], tri_d[:])
            bq_sb = consts.tile([128, 2], f32)
            bk_sb = consts.tile([128, 2], f32)
            bv_sb = consts.tile([64, HLOC], f32)
            nc.sync.dma_start(bq_sb[:], bq_d.rearrange("(ct p) -> p ct", p=128))
            nc.sync.dma_start(bk_sb[:], bk_d.rearrange("(ct p) -> p ct", p=128))
            nc.sync.dma_start(bv_sb[:], bv_d[:])

            wq = consts.tile([128, MO, CLOC], bf16)
            wk = consts.tile([128, MO, CLOC], bf16)
            wv = consts.tile([128, MO, CLOC], bf16)
            nc.sync.dma_start(wq[:], wq_d.rearrange("(mo p) c -> p mo c", p=128))
            nc.sync.dma_start(wk[:], wk_d.rearrange("(mo p) c -> p mo c", p=128))
            nc.sync.dma_start(wv[:], wv_d.rearrange("(mo p) c -> p mo c", p=128))
            wo = consts.tile([128, 2, DM], bf16)
            nc.sync.dma_start(wo[:], wo_d.rearrange("(co p) n -> p co n", p=128))

            xT = apool.tile([128, MO, S], bf16)
            nc.sync.dma_start(xT[:], xT_d.rearrange("(mo p) t -> p mo t", p=128))

            QT = apool.tile([128, 2, S], bf16)
            KT = apool.tile([128, 2, S], bf16)
            # V augmented: [t-part, kt, h, 0:64] = v dims, col 64 = ones
            VA = apool.tile([128, NKT, HLOC, 72], bf16)
            nc.vector.memset(VA[:, :, :, 64:65], 1.0)
            zT = apool.tile([128, 2, S], bf16)

            # ---- phase 1: QKV projections --------------------------------
            for tg in range(S // 512):
                tsl = slice(tg * 512, (tg + 1) * 512)
                for ct in range(2):
                    csl = slice(ct * 128, (ct + 1) * 128)
                    for w_sb, dst, b_sb in ((wq, QT, bq_sb), (wk, KT, bk_sb)):
                        ps = ps_s.tile([128, 1024], f32, tag="s")
                        for mo in range(MO):
                            nc.tensor.matmul(
                                ps[:, 0:512],
                                w_sb[:, mo, csl],
                                xT[:, mo, tsl],
                                start=(mo == 0),
                                stop=(mo == MO - 1),
                            )
                        nc.vector.tensor_scalar_add(
                            dst[:, ct, tsl], ps[:, 0:512], b_sb[:, ct : ct + 1]
                        )
                # V: out [t-128, 256] per t-tile
                for ti in range(4):
                    tt = tg * 4 + ti
                    ps = ps_z.tile([128, 512], f32, tag="z")
                    for mo in range(MO):
                        nc.tensor.matmul(
                            ps[:, 0:CLOC],
                            xT[:, mo, tt * 128 : (tt + 1) * 128],
                            wv[:, mo, :],
                            start=(mo == 0),
                            stop=(mo == MO - 1),
                        )
                    nc.vector.tensor_copy(
                        VA[:, tt, :, 0:64],
                        ps[:, 0:CLOC].rearrange("p (h d) -> p h d", d=64),
                    )

            # ---- phase 2: attention --------------------------------------
            for h in range(HLOC):
                hp = (h % 2) * 64
                ct = h // 2
                for qh in range(2):
                    h0 = qh * 1024
                    hend = (qh + 1) * 1024
                    zps = [
                        ps_z.tile([128, 512], f32, tag="z", name=f"zps_{h}_{qh}_{g}")
                        for g in range(2)
                    ]
                    for kt in range((qh + 1) * 8):
                        q0 = max(kt * 128, h0)
                        w = hend - q0
                        sreg = ps_s.tile([128, 1024], f32, tag="s")
                        for c0 in range(0, w, 512):
                            cw = min(512, w - c0)
                            nc.tensor.matmul(
                                sreg[:, c0 : c0 + cw],
                                KT[hp : hp + 64, ct, kt * 128 : (kt + 1) * 128],
                                QT[hp : hp + 64, ct, q0 + c0 : q0 + c0 + cw],
                            )
                        if kt * 128 >= h0:  # diagonal block leads the span
                            nc.vector.tensor_add(
                                sreg[:, 0:128], sreg[:, 0:128], tri[:]
                            )
                        pT = ppool.tile([128, 1024], bf16, tag="pT")
                        nc.scalar.activation(
                            pT[:, :w], sreg[:, :w], EXP, scale=0.125
                        )
                        for qg in range(2):
                            g0 = h0 + qg * 512
                            if g0 + 512 <= q0:
                                continue
                            a0 = max(q0, g0)
                            nc.tensor.matmul(
                                zps[qg][0:65, a0 - g0 : 512],
                                VA[:, kt, h, 0:65],
                                pT[:, a0 - q0 : g0 + 512 - q0],
                                start=(kt == 0),
                                stop=(kt == (g0 + 512) // 128 - 1),
                            )
                    # normalize the two quarters of this half
                    for qg in range(2):
                        g0 = h0 + qg * 512
                        rec32 = spool.tile([1, 512], f32, tag="rec32")
                        nc.vector.reciprocal(rec32[:], zps[qg][64:65, 0:512])
                        bcast = spool.tile([64, 512], f32, tag="bcast")
                        nc.gpsimd.partition_broadcast(bcast[:], rec32[:])
                        zmul = spool.tile([64, 512], f32, tag="zmul")
                        nc.vector.tensor_mul(
                            zmul[:], zps[qg][0:64, 0:512], bcast[:]
                        )
                        zbf = spool.tile([64, 512], bf16, tag="zbf")
                        nc.vector.tensor_scalar_add(
                            zbf[:], zmul[:], bv_sb[:, h : h + 1]
                        )
                        nc.sync.dma_start(
                            zT[hp : hp + 64, ct, g0 : g0 + 512], zbf[:]
                        )

            # ---- phase 3: output projection ------------------------------
            for tt in range(NKT):
                for nh in range(2):
                    ps = ps_b.tile([128, 512], f32, tag="b")
                    for co in range(2):
                        nc.tensor.matmul(
                            ps[:],
                            zT[:, co, tt * 128 : (tt + 1) * 128],
                            wo[:, co, nh * 512 : (nh + 1) * 512],
                            start=(co == 0),
                            stop=(co == 1),
                        )
                    ysb = ypool.tile([128, 512], f32, tag="y")
                    if (tt + nh) % 2 == 0:
                        nc.scalar.copy(ysb[:], ps[:])
                    else:
                        nc.vector.tensor_copy(ysb[:], ps[:])
                    nc.sync.dma_start(
                        y_d[tt * 128 : (tt + 1) * 128, nh * 512 : (nh + 1) * 512],
                        ysb[:],
                    )

    nc.compile()
    return nc


def make_in_maps(x, w_qkv, b_qkv, w_out):
    tri = np.where(np.tri(128, 128, 0, dtype=bool).T, 0.0, NEG).astype(np.float32)
    in_maps = []
    for core in range(8):
        b = core // 4
        hg = core % 4
        c0 = hg * CLOC
        csl = slice(c0, c0 + CLOC)
        bv = b_qkv[2 * DM :][csl].astype(np.float32)
        in_maps.append(
            {
                "xT": np.ascontiguousarray(x[b].T).astype(ml_dtypes.bfloat16),
                "wq": np.ascontiguousarray(w_qkv[:, csl]).astype(ml_dtypes.bfloat16),
                "wk": np.ascontiguousarray(w_qkv[:, DM + c0 : DM + c0 + CLOC]).astype(
                    ml_dtypes.bfloat16
                ),
                "wv": np.ascontiguousarray(
                    w_qkv[:, 2 * DM + c0 : 2 * DM + c0 + CLOC]
                ).astype(ml_dtypes.bfloat16),
                "wo": np.ascontiguousarray(w_out[csl, :]).astype(ml_dtypes.bfloat16),
                "bq": b_qkv[csl].astype(np.float32),
                "bk": b_qkv[DM + c0 : DM + c0 + CLOC].astype(np.float32),
                # bv indexed [d, h] so device-side slices sit at partition 0
                "bv": np.ascontiguousarray(bv.reshape(HLOC, 64).T),
                "tri": tri,
            }
        )
    return in_maps


def gather(results, b_out):
    out = np.empty((B, S, DM), np.float32)
    for b in range(B):
        acc = results[4 * b]["y"].astype(np.float32)
        for j in range(1, 4):
            acc = acc + results[4 * b + j]["y"]
        out[b] = acc + b_out.astype(np.float32)[None, :]
    return out


def kernel(x, w_qkv, b_qkv, w_out, b_out):
    x = np.asarray(x)
    w_qkv = np.asarray(w_qkv)
    b_qkv = np.asarray(b_qkv)
    w_out = np.asarray(w_out)
    b_out = np.asarray(b_out)

    if "nc" not in _CACHE:
        _CACHE["nc"] = build()
    nc = _CACHE["nc"]

    in_maps = make_in_maps(x, w_qkv, b_qkv, w_out)
    res = run_bass_kernel_spmd(nc, in_maps, core_ids=list(range(8)))
    return gather(res.results, b_out)


# revision 40
# speedup vs baseline: 421.5231x; 421.5231x over previous
"""Causal multi-head attention block (B=2, S=2048, D=1024, H=16) on 8 TRN2 cores.

Sharding: core i handles batch b = i//4 and head group hg = i%4 (4 heads =
256 model dims). Each core computes its heads' attention and a partial
output projection; the host sums the 4 partials per batch and adds b_out.

Per-core device pipeline (bf16 matmuls, fp32 PSUM accumulation):
  1. QKV. Q^T,K^T land as [head_cols, tokens] (lhsT = W, rhs = x^T);
     V lands as [tokens, head_cols] (lhsT = x^T tiles, rhs = W_v) and is
     stored augmented with a ones column so the attention z-matmul also
     produces softmax row sums.
  2. Attention per head, flash-style in the S^T = K.Q^T orientation over
     the causal lower triangle only: S^T[k_tile, q_span] -> exp on ScalarE
     (scale=1/8; no max subtraction, logits are ~N(0,1)) -> P^T bf16 ->
     multiplicative 0/1 mask on the diagonal block -> z^T[d+1, q] +=
     V_aug^T @ P^T accumulated over k tiles in PSUM. Consecutive k tiles
     share one S region so each exp call covers up to 1024 columns.
  3. Normalize as soon as a q-quarter's last k tile lands: recip(rowsum),
     GPSIMD partition-broadcast, z * recip + b_v on VectorE -> bf16 zT.
  4. Out-proj: y_partial[t, n] accumulated over the 256 local dims.

Program order interleaves token halves -- QKV(cols 0:1024), attention
(q 0:1024), QKV(cols 1024:2048), attention(q 1024:2048), out-proj -- so
ScalarE exp work overlaps PE QKV work.
"""

import numpy as np
import ml_dtypes

import concourse.mybir as mybir
import concourse.tile as tile
from concourse import bacc
from concourse.bass_utils import run_bass_kernel_spmd

B = 2
S = 2048
DM = 1024
HD = 64
HLOC = 4                 # heads per core
CLOC = HLOC * HD         # local model dims (256)
MO = DM // 128           # 8 k-subtiles of the model dim
NKT = S // 128           # 16 key tiles


f32 = mybir.dt.float32
bf16 = mybir.dt.bfloat16
EXP = mybir.ActivationFunctionType.Exp

_CACHE = {}


def build(ps_z_bufs=2, pt_bufs=5, op_engines=("dve", "dve", "dve", "mix"),
          interleave_heads=False):
    nc = bacc.Bacc("TRN2", target_bir_lowering=False, debug=False)

    xT_d = nc.dram_tensor("xT", [128, MO, S], bf16, kind="ExternalInput")
    wqk_d = nc.dram_tensor("wqk", [128, MO, 2 * CLOC], bf16, kind="ExternalInput")
    wv_d = nc.dram_tensor("wv", [128, MO, CLOC], bf16, kind="ExternalInput")
    wo_d = nc.dram_tensor("wo", [128, 2, DM], bf16, kind="ExternalInput")
    # consts packed as raw bf16 columns: tri[0:128], bq[128:132],
    # bk[132:136], bv[136:144] (f32 values bit-split across bf16 pairs)
    cst_d = nc.dram_tensor("cst", [128, 144], bf16, kind="ExternalInput")
    y_d = nc.dram_tensor("y", [S, DM], f32, kind="ExternalOutput")

    with tile.TileContext(nc) as tc:
        with (
            tc.tile_pool(name="consts", bufs=1) as consts,
            tc.tile_pool(name="acts", bufs=1) as apool,
            tc.tile_pool(name="pt", bufs=pt_bufs) as ppool,
            tc.tile_pool(name="norm", bufs=4) as spool,
            tc.tile_pool(name="ycopy", bufs=4) as ypool,
            # 8 PSUM banks: ps_s 2x[128,1024]=4 (QKV Q/K + attention S),
            # ps_z [128,512] z accumulators, ps_b rest (V / out-proj)
            tc.tile_pool(name="ps_s", bufs=2, space="PSUM") as ps_s,
            tc.tile_pool(name="ps_z", bufs=ps_z_bufs, space="PSUM") as ps_z,
            tc.tile_pool(name="ps_b", bufs=8 - 4 - ps_z_bufs, space="PSUM") as ps_b,
        ):
            csb = consts.tile([128, 144], bf16)
            wqk = consts.tile([128, MO, 2 * CLOC], bf16)
            wv = consts.tile([128, MO, CLOC], bf16)
            wo = consts.tile([128, 2, DM], bf16)
            xT = apool.tile([128, MO, S], bf16)

            # DMA order = consumption order. First chunks are fine-grained
            # so the first QKV matmul starts ~2.5us in; the rest are big
            # transfers to minimize per-DMA descriptor overhead.
            nc.sync.dma_start(wqk[:, 0:2, :], wqk_d[:, 0:2, :])
            nc.scalar.dma_start(xT[:, 0:2, 0:512], xT_d[:, 0:2, 0:512])
            nc.sync.dma_start(csb[:], cst_d[:])
            nc.scalar.dma_start(wqk[:, 2:MO, :], wqk_d[:, 2:MO, :])
            nc.sync.dma_start(xT[:, 2:MO, 0:512], xT_d[:, 2:MO, 0:512])
            nc.scalar.dma_start(wv[:], wv_d[:])
            nc.sync.dma_start(xT[:, :, 512:1024], xT_d[:, :, 512:1024])
            nc.scalar.dma_start(xT[:, :, 1024:1536], xT_d[:, :, 1024:1536])
            nc.sync.dma_start(xT[:, :, 1536:2048], xT_d[:, :, 1536:2048])
            nc.scalar.dma_start(wo[:], wo_d[:])

            tri = csb[:, 0:128]
            bq_sb = csb[:, 128:132].bitcast(f32)
            bk_sb = csb[:, 132:136].bitcast(f32)
            bv_sb = csb[0:64, 136:144].bitcast(f32)

            QT = apool.tile([128, 2, S], bf16)
            KT = apool.tile([128, 2, S], bf16)
            # V augmented: [t-part, kt, h, 0:64] = v dims, col 64 = ones
            VA = apool.tile([128, NKT, HLOC, 72], bf16)
            nc.vector.memset(VA[:, :, :, 64:65], 1.0)
            zT = apool.tile([128, 2, S], bf16)

            def emit_qkv(tg):
                tsl = slice(tg * 512, (tg + 1) * 512)
                for ct in range(2):
                    for wof, dst, b_sb in ((0, QT, bq_sb), (CLOC, KT, bk_sb)):
                        csl = slice(wof + ct * 128, wof + (ct + 1) * 128)
                        ps = ps_s.tile([128, 1024], f32, tag="s")
                        for mo in range(MO):
                            nc.tensor.matmul(
                                ps[:, 0:512],
                                wqk[:, mo, csl],
                                xT[:, mo, tsl],
                                start=(mo == 0),
                                stop=(mo == MO - 1),
                            )
                        nc.vector.tensor_scalar_add(
                            dst[:, ct, tsl], ps[:, 0:512], b_sb[:, ct : ct + 1]
                        )
                for ti in range(4):
                    tt = tg * 4 + ti
                    ps = ps_b.tile([128, 512], f32, tag="b")
                    for mo in range(MO):
                        nc.tensor.matmul(
                            ps[:, 0:CLOC],
                            xT[:, mo, tt * 128 : (tt + 1) * 128],
                            wv[:, mo, :],
                            start=(mo == 0),
                            stop=(mo == MO - 1),
                        )
                    if tg == 0:
                        nc.scalar.copy(
                            VA[:, tt, :, 0:64],
                            ps[:, 0:CLOC].rearrange("p (h d) -> p h d", d=64),
                        )
                    else:
                        nc.vector.tensor_copy(
                            VA[:, tt, :, 0:64],
                            ps[:, 0:CLOC].rearrange("p (h d) -> p h d", d=64),
                        )

            def emit_attention(qg):
                g0 = qg * 512
                last_kt = 4 * qg + 3

                # pack consecutive k tiles into shared S regions so one
                # exp call covers up to 1024 columns
                groups, cur, cum = [], [], 0
                for kt in range(last_kt + 1):
                    w = g0 + 512 - max(kt * 128, g0)
                    if cum + w > 1024:
                        groups.append(cur)
                        cur, cum = [], 0
                    cur.append((kt, cum, w))
                    cum += w
                groups.append(cur)

                head_lists = ([0, 1, 2, 3] if not interleave_heads
                              else [[0, 1], [2, 3]])
                for hl in head_lists:
                    hs = [hl] if isinstance(hl, int) else hl
                    zp_map = {}
                    for h in hs:
                        zp_map[h] = ps_z.tile([128, 512], f32, tag="z",
                                              name=f"zps_{h}_{qg}")
                    for grp in groups:
                        for h in hs:
                            emit_head_grp(h, grp, zp_map[h], qg, g0, last_kt)
                    for h in hs:
                        emit_norm(h, zp_map[h], qg, g0)

            def emit_head_grp(h, grp, zp, qg, g0, last_kt):
                    hp = (h % 2) * 64
                    ct = h // 2
                    if True:
                        sreg = ps_s.tile([128, 1024], f32, tag="s",
                                         name=f"s_{h}_{qg}_{grp[0][0]}")
                        cum = grp[-1][1] + grp[-1][2]
                        for kt, off, w in grp:
                            q0 = g0 + 512 - w
                            c0 = off
                            while c0 < off + w:
                                cw = min(off + w - c0, 512 - c0 % 512)
                                nc.tensor.matmul(
                                    sreg[:, c0 : c0 + cw],
                                    KT[hp : hp + 64, ct,
                                       kt * 128 : (kt + 1) * 128],
                                    QT[hp : hp + 64, ct,
                                       q0 + c0 - off : q0 + c0 - off + cw],
                                )
                                c0 += cw
                        pT = ppool.tile([128, 1024], bf16, tag="pT")
                        nc.scalar.activation(
                            pT[:, :cum], sreg[:, :cum], EXP, scale=0.125
                        )
                        for kt, off, w in grp:
                            if kt * 128 >= g0:  # diagonal block leads span
                                nc.vector.tensor_mul(
                                    pT[:, off : off + 128],
                                    pT[:, off : off + 128],
                                    tri[:],
                                )
                            q0 = g0 + 512 - w
                            nc.tensor.matmul(
                                zp[0:65, q0 - g0 : 512],
                                VA[:, kt, h, 0:65],
                                pT[:, off : off + w],
                                start=(kt == 0),
                                stop=(kt == last_kt),
                            )

            def emit_norm(h, zp, qg, g0):
                    hp = (h % 2) * 64
                    ct = h // 2
                    rec32 = spool.tile([1, 512], f32, tag="rec32",
                                       name=f"rec_{h}_{qg}")
                    nc.vector.reciprocal(rec32[:], zp[64:65, 0:512])
                    bcast = spool.tile([64, 512], f32, tag="bcast",
                                       name=f"bc_{h}_{qg}")
                    nc.gpsimd.partition_broadcast(bcast[:], rec32[:])
                    zmul = spool.tile([64, 512], f32, tag="zmul",
                                      name=f"zm_{h}_{qg}")
                    nc.vector.tensor_mul(zmul[:], zp[0:64, 0:512], bcast[:])
                    if hp == 0:
                        nc.vector.tensor_scalar_add(
                            zT[0:64, ct, g0 : g0 + 512], zmul[:],
                            bv_sb[:, h : h + 1],
                        )
                    else:
                        zbf = spool.tile([64, 512], bf16, tag="zbf",
                                         name=f"zb_{h}_{qg}")
                        nc.vector.tensor_scalar_add(
                            zbf[:], zmul[:], bv_sb[:, h : h + 1]
                        )
                        nc.sync.dma_start(
                            zT[hp : hp + 64, ct, g0 : g0 + 512], zbf[:]
                        )

            def emit_outproj(qg, copy_eng, dma_split=1):
                for nh in range(2):
                    ysb = ypool.tile([128, 4, 512], f32, tag="y",
                                     name=f"ysb_{qg}_{nh}")
                    nper = 4 // dma_split
                    for ti in range(4):
                        tt = qg * 4 + ti
                        ps = ps_b.tile([128, 512], f32, tag="b")
                        for co in range(2):
                            nc.tensor.matmul(
                                ps[:],
                                zT[:, co, tt * 128 : (tt + 1) * 128],
                                wo[:, co, nh * 512 : (nh + 1) * 512],
                                start=(co == 0),
                                stop=(co == 1),
                            )
                        eng = copy_eng if copy_eng != "mix" else (
                            "act" if (tt + nh) % 2 == 0 else "dve"
                        )
                        if eng == "act":
                            nc.scalar.copy(ysb[:, ti, :], ps[:])
                        else:
                            nc.vector.tensor_copy(ysb[:, ti, :], ps[:])
                        if ti % nper == nper - 1:
                            t0 = tt - nper + 1
                            nc.sync.dma_start(
                                y_d[t0 * 128 : (tt + 1) * 128,
                                    nh * 512 : (nh + 1) * 512].rearrange(
                                    "(ti p) n -> p ti n", p=128
                                ),
                                ysb[:, ti - nper + 1 : ti + 1, :],
                            )

            # 4-stage software pipeline: attention on quarter qg overlaps
            # the QKV projection of quarter qg+1 on PE
            emit_qkv(0)
            emit_attention(0)
            emit_qkv(1)
            emit_attention(1)
            emit_qkv(2)
            emit_attention(2)
            emit_qkv(3)
            emit_attention(3)
            for qg in range(4):
                emit_outproj(qg, op_engines[qg], dma_split=2 if qg == 3 else 1)

    nc.compile()
    return nc


def _pack_w(w):
    # [DM, C] -> [128, MO, C]: partition p holds rows {mo*128 + p}
    return np.ascontiguousarray(
        w.reshape(MO, 128, w.shape[1]).transpose(1, 0, 2)
    ).astype(ml_dtypes.bfloat16)


def make_in_maps(x, w_qkv, b_qkv, w_out):
    # multiplicative post-exp mask: 1 where k <= q (upper incl diag), else 0
    tri = np.tri(128, 128, 0, dtype=np.float32).T.astype(ml_dtypes.bfloat16)
    in_maps = []
    for core in range(8):
        b = core // 4
        hg = core % 4
        c0 = hg * CLOC
        csl = slice(c0, c0 + CLOC)

        # packed consts: [128, 144] bf16-typed raw columns
        cst = np.zeros((128, 144), np.uint16)
        cst[:, 0:128] = tri.view(np.uint16)
        bq = np.ascontiguousarray(
            b_qkv[csl].astype(np.float32).reshape(2, 128).T
        )
        bk = np.ascontiguousarray(
            b_qkv[DM + c0 : DM + c0 + CLOC].astype(np.float32).reshape(2, 128).T
        )
        bv = np.ascontiguousarray(
            b_qkv[2 * DM + c0 : 2 * DM + c0 + CLOC]
            .astype(np.float32).reshape(HLOC, 64).T
        )
        cst[:, 128:132] = bq.view(np.uint16).reshape(128, 4)
        cst[:, 132:136] = bk.view(np.uint16).reshape(128, 4)
        cst[0:64, 136:144] = bv.view(np.uint16).reshape(64, 8)

        wqk = np.concatenate(
            [_pack_w(w_qkv[:, csl]), _pack_w(w_qkv[:, DM + c0 : DM + c0 + CLOC])],
            axis=2,
        )
        in_maps.append(
            {
                "xT": _pack_w(np.ascontiguousarray(x[b].T)),
                "wqk": np.ascontiguousarray(wqk),
                "wv": _pack_w(w_qkv[:, 2 * DM + c0 : 2 * DM + c0 + CLOC]),
                # wo: [CLOC, DM] -> [128, 2, DM]
                "wo": np.ascontiguousarray(
                    w_out[csl, :].reshape(2, 128, DM).transpose(1, 0, 2)
                ).astype(ml_dtypes.bfloat16),
                "cst": cst.view(ml_dtypes.bfloat16),
            }
        )
    return in_maps


def gather(results, b_out):
    out = np.empty((B, S, DM), np.float32)
    for b in range(B):
        acc = results[4 * b]["y"].astype(np.float32)
        for j in range(1, 4):
            acc = acc + results[4 * b + j]["y"]
        out[b] = acc + b_out.astype(np.float32)[None, :]
    return out


def kernel(x, w_qkv, b_qkv, w_out, b_out):
    x = np.asarray(x)
    w_qkv = np.asarray(w_qkv)
    b_qkv = np.asarray(b_qkv)
    w_out = np.asarray(w_out)
    b_out = np.asarray(b_out)

    if "nc" not in _CACHE:
        _CACHE["nc"] = build()
    nc = _CACHE["nc"]

    in_maps = make_in_maps(x, w_qkv, b_qkv, w_out)
    res = run_bass_kernel_spmd(nc, in_maps, core_ids=list(range(8)))
    return gather(res.results, b_out)


# revision 41
# speedup vs baseline: 423.8348x; 1.0055x over previous
"""Causal multi-head attention block (B=2, S=2048, D=1024, H=16) on 8 TRN2 cores.

Sharding: core i handles batch b = i//4 and head group hg = i%4 (4 heads =
256 model dims). Each core computes its heads' attention and a partial
output projection; the host sums the 4 partials per batch and adds b_out.

Per-core device pipeline (bf16 matmuls, fp32 PSUM accumulation):
  1. QKV. Q^T,K^T land as [head_cols, tokens] (lhsT = W, rhs = x^T);
     V lands as [tokens, head_cols] (lhsT = x^T tiles, rhs = W_v) and is
     stored augmented with a ones column so the attention z-matmul also
     produces softmax row sums.
  2. Attention per head, flash-style in the S^T = K.Q^T orientation over
     the causal lower triangle only: S^T[k_tile, q_span] -> exp on ScalarE
     (scale=1/8; no max subtraction, logits are ~N(0,1)) -> P^T bf16 ->
     multiplicative 0/1 mask on the diagonal block -> z^T[d+1, q] +=
     V_aug^T @ P^T accumulated over k tiles in PSUM. Consecutive k tiles
     share one S region so each exp call covers up to 1024 columns.
  3. Normalize as soon as a q-quarter's last k tile lands: recip(rowsum),
     GPSIMD partition-broadcast, z * recip + b_v on VectorE -> bf16 zT.
  4. Out-proj: y_partial[t, n] accumulated over the 256 local dims.

Program order interleaves token halves -- QKV(cols 0:1024), attention
(q 0:1024), QKV(cols 1024:2048), attention(q 1024:2048), out-proj -- so
ScalarE exp work overlaps PE QKV work.
"""

import numpy as np
import ml_dtypes

import concourse.mybir as mybir
import concourse.tile as tile
from concourse import bacc
from concourse.bass_utils import run_bass_kernel_spmd

B = 2
S = 2048
DM = 1024
HD = 64
HLOC = 4                 # heads per core
CLOC = HLOC * HD         # local model dims (256)
MO = DM // 128           # 8 k-subtiles of the model dim
NKT = S // 128           # 16 key tiles


f32 = mybir.dt.float32
bf16 = mybir.dt.bfloat16
EXP = mybir.ActivationFunctionType.Exp

_CACHE = {}


def build(ps_z_bufs=2, pt_bufs=5, op_engines=("dve", "dve", "dve", "mix"),
          interleave_heads=False):
    nc = bacc.Bacc("TRN2", target_bir_lowering=False, debug=False)

    xT_d = nc.dram_tensor("xT", [128, MO, S], bf16, kind="ExternalInput")
    wqk_d = nc.dram_tensor("wqk", [128, MO, 2 * CLOC], bf16, kind="ExternalInput")
    wv_d = nc.dram_tensor("wv", [128, MO, CLOC], bf16, kind="ExternalInput")
    wo_d = nc.dram_tensor("wo", [128, 2, DM], bf16, kind="ExternalInput")
    # consts packed as raw bf16 columns: tri[0:128], bq[128:132],
    # bk[132:136], bv[136:144] (f32 values bit-split across bf16 pairs)
    cst_d = nc.dram_tensor("cst", [128, 144], bf16, kind="ExternalInput")
    y_d = nc.dram_tensor("y", [S, DM], f32, kind="ExternalOutput")

    with tile.TileContext(nc) as tc:
        with (
            tc.tile_pool(name="consts", bufs=1) as consts,
            tc.tile_pool(name="acts", bufs=1) as apool,
            tc.tile_pool(name="pt", bufs=pt_bufs) as ppool,
            tc.tile_pool(name="norm", bufs=4) as spool,
            tc.tile_pool(name="ycopy", bufs=4) as ypool,
            # 8 PSUM banks: ps_s 2x[128,1024]=4 (QKV Q/K + attention S),
            # ps_z [128,512] z accumulators, ps_b rest (V / out-proj)
            tc.tile_pool(name="ps_s", bufs=2, space="PSUM") as ps_s,
            tc.tile_pool(name="ps_z", bufs=ps_z_bufs, space="PSUM") as ps_z,
            tc.tile_pool(name="ps_b", bufs=8 - 4 - ps_z_bufs, space="PSUM") as ps_b,
        ):
            csb = consts.tile([128, 144], bf16)
            wqk = consts.tile([128, MO, 2 * CLOC], bf16)
            wv = consts.tile([128, MO, CLOC], bf16)
            wo = consts.tile([128, 2, DM], bf16)
            xT = apool.tile([128, MO, S], bf16)

            # DMA order = consumption order. First chunks are fine-grained
            # so the first QKV matmul starts ~2.5us in; the rest are big
            # transfers to minimize per-DMA descriptor overhead.
            nc.sync.dma_start(wqk[:, 0:3, :], wqk_d[:, 0:3, :])
            nc.scalar.dma_start(xT[:, 0:3, 0:512], xT_d[:, 0:3, 0:512])
            nc.sync.dma_start(csb[:], cst_d[:])
            nc.scalar.dma_start(wqk[:, 3:MO, :], wqk_d[:, 3:MO, :])
            nc.sync.dma_start(xT[:, 3:MO, 0:512], xT_d[:, 3:MO, 0:512])
            nc.scalar.dma_start(wv[:], wv_d[:])
            nc.sync.dma_start(xT[:, :, 512:1024], xT_d[:, :, 512:1024])
            nc.scalar.dma_start(xT[:, :, 1024:1536], xT_d[:, :, 1024:1536])
            nc.sync.dma_start(xT[:, :, 1536:2048], xT_d[:, :, 1536:2048])
            nc.scalar.dma_start(wo[:], wo_d[:])

            tri = csb[:, 0:128]
            bq_sb = csb[:, 128:132].bitcast(f32)
            bk_sb = csb[:, 132:136].bitcast(f32)
            bv_sb = csb[0:64, 136:144].bitcast(f32)

            QT = apool.tile([128, 2, S], bf16)
            KT = apool.tile([128, 2, S], bf16)
            # V augmented: [t-part, kt, h, 0:64] = v dims, col 64 = ones
            VA = apool.tile([128, NKT, HLOC, 72], bf16)
            nc.vector.memset(VA[:, :, :, 64:65], 1.0)
            zT = apool.tile([128, 2, S], bf16)

            def emit_qkv(tg):
                tsl = slice(tg * 512, (tg + 1) * 512)
                for ct in range(2):
                    for wof, dst, b_sb in ((0, QT, bq_sb), (CLOC, KT, bk_sb)):
                        csl = slice(wof + ct * 128, wof + (ct + 1) * 128)
                        ps = ps_s.tile([128, 1024], f32, tag="s")
                        for mo in range(MO):
                            nc.tensor.matmul(
                                ps[:, 0:512],
                                wqk[:, mo, csl],
                                xT[:, mo, tsl],
                                start=(mo == 0),
                                stop=(mo == MO - 1),
                            )
                        nc.vector.tensor_scalar_add(
                            dst[:, ct, tsl], ps[:, 0:512], b_sb[:, ct : ct + 1]
                        )
                for ti in range(4):
                    tt = tg * 4 + ti
                    ps = ps_b.tile([128, 512], f32, tag="b")
                    for mo in range(MO):
                        nc.tensor.matmul(
                            ps[:, 0:CLOC],
                            xT[:, mo, tt * 128 : (tt + 1) * 128],
                            wv[:, mo, :],
                            start=(mo == 0),
                            stop=(mo == MO - 1),
                        )
                    if tg == 0:
                        nc.scalar.copy(
                            VA[:, tt, :, 0:64],
                            ps[:, 0:CLOC].rearrange("p (h d) -> p h d", d=64),
                        )
                    else:
                        nc.vector.tensor_copy(
                            VA[:, tt, :, 0:64],
                            ps[:, 0:CLOC].rearrange("p (h d) -> p h d", d=64),
                        )

            def emit_attention(qg):
                g0 = qg * 512
                last_kt = 4 * qg + 3

                # pack consecutive k tiles into shared S regions so one
                # exp call covers up to 1024 columns
                groups, cur, cum = [], [], 0
                for kt in range(last_kt + 1):
                    w = g0 + 512 - max(kt * 128, g0)
                    if cum + w > 1024:
                        groups.append(cur)
                        cur, cum = [], 0
                    cur.append((kt, cum, w))
                    cum += w
                groups.append(cur)

                head_lists = ([0, 1, 2, 3] if not interleave_heads
                              else [[0, 1], [2, 3]])
                for hl in head_lists:
                    hs = [hl] if isinstance(hl, int) else hl
                    zp_map = {}
                    for h in hs:
                        zp_map[h] = ps_z.tile([128, 512], f32, tag="z",
                                              name=f"zps_{h}_{qg}")
                    for grp in groups:
                        for h in hs:
                            emit_head_grp(h, grp, zp_map[h], qg, g0, last_kt)
                    for h in hs:
                        emit_norm(h, zp_map[h], qg, g0)

            def emit_head_grp(h, grp, zp, qg, g0, last_kt):
                    hp = (h % 2) * 64
                    ct = h // 2
                    if True:
                        sreg = ps_s.tile([128, 1024], f32, tag="s",
                                         name=f"s_{h}_{qg}_{grp[0][0]}")
                        cum = grp[-1][1] + grp[-1][2]
                        for kt, off, w in grp:
                            q0 = g0 + 512 - w
                            c0 = off
                            while c0 < off + w:
                                cw = min(off + w - c0, 512 - c0 % 512)
                                nc.tensor.matmul(
                                    sreg[:, c0 : c0 + cw],
                                    KT[hp : hp + 64, ct,
                                       kt * 128 : (kt + 1) * 128],
                                    QT[hp : hp + 64, ct,
                                       q0 + c0 - off : q0 + c0 - off + cw],
                                )
                                c0 += cw
                        pT = ppool.tile([128, 1024], bf16, tag="pT")
                        nc.scalar.activation(
                            pT[:, :cum], sreg[:, :cum], EXP, scale=0.125
                        )
                        for kt, off, w in grp:
                            if kt * 128 >= g0:  # diagonal block leads span
                                nc.vector.tensor_mul(
                                    pT[:, off : off + 128],
                                    pT[:, off : off + 128],
                                    tri[:],
                                )
                            q0 = g0 + 512 - w
                            nc.tensor.matmul(
                                zp[0:65, q0 - g0 : 512],
                                VA[:, kt, h, 0:65],
                                pT[:, off : off + w],
                                start=(kt == 0),
                                stop=(kt == last_kt),
                            )

            def emit_norm(h, zp, qg, g0):
                    hp = (h % 2) * 64
                    ct = h // 2
                    rec32 = spool.tile([1, 512], f32, tag="rec32",
                                       name=f"rec_{h}_{qg}")
                    nc.vector.reciprocal(rec32[:], zp[64:65, 0:512])
                    bcast = spool.tile([64, 512], f32, tag="bcast",
                                       name=f"bc_{h}_{qg}")
                    nc.gpsimd.partition_broadcast(bcast[:], rec32[:])
                    zmul = spool.tile([64, 512], f32, tag="zmul",
                                      name=f"zm_{h}_{qg}")
                    nc.vector.tensor_mul(zmul[:], zp[0:64, 0:512], bcast[:])
                    if hp == 0:
                        nc.vector.tensor_scalar_add(
                            zT[0:64, ct, g0 : g0 + 512], zmul[:],
                            bv_sb[:, h : h + 1],
                        )
                    else:
                        zbf = spool.tile([64, 512], bf16, tag="zbf",
                                         name=f"zb_{h}_{qg}")
                        nc.vector.tensor_scalar_add(
                            zbf[:], zmul[:], bv_sb[:, h : h + 1]
                        )
                        nc.sync.dma_start(
                            zT[hp : hp + 64, ct, g0 : g0 + 512], zbf[:]
                        )

            def emit_outproj(qg, copy_eng, dma_split=1):
                for nh in range(2):
                    ysb = ypool.tile([128, 4, 512], f32, tag="y",
                                     name=f"ysb_{qg}_{nh}")
                    nper = 4 // dma_split
                    for ti in range(4):
                        tt = qg * 4 + ti
                        ps = ps_b.tile([128, 512], f32, tag="b")
                        for co in range(2):
                            nc.tensor.matmul(
                                ps[:],
                                zT[:, co, tt * 128 : (tt + 1) * 128],
                                wo[:, co, nh * 512 : (nh + 1) * 512],
                                start=(co == 0),
                                stop=(co == 1),
                            )
                        eng = copy_eng if copy_eng != "mix" else (
                            "act" if (tt + nh) % 2 == 0 else "dve"
                        )
                        if eng == "act":
                            nc.scalar.copy(ysb[:, ti, :], ps[:])
                        else:
                            nc.vector.tensor_copy(ysb[:, ti, :], ps[:])
                        if ti % nper == nper - 1:
                            t0 = tt - nper + 1
                            nc.sync.dma_start(
                                y_d[t0 * 128 : (tt + 1) * 128,
                                    nh * 512 : (nh + 1) * 512].rearrange(
                                    "(ti p) n -> p ti n", p=128
                                ),
                                ysb[:, ti - nper + 1 : ti + 1, :],
                            )

            # 4-stage software pipeline: attention on quarter qg overlaps
            # the QKV projection of quarter qg+1 on PE
            emit_qkv(0)
            emit_attention(0)
            emit_qkv(1)
            emit_attention(1)
            emit_qkv(2)
            emit_attention(2)
            emit_qkv(3)
            emit_attention(3)
            for qg in range(4):
                emit_outproj(qg, op_engines[qg], dma_split=2 if qg == 3 else 1)

    nc.compile()
    return nc


def _pack_w(w):
    # [DM, C] -> [128, MO, C]: partition p holds rows {mo*128 + p}
    return np.ascontiguousarray(
        w.reshape(MO, 128, w.shape[1]).transpose(1, 0, 2)
    ).astype(ml_dtypes.bfloat16)


def make_in_maps(x, w_qkv, b_qkv, w_out):
    # multiplicative post-exp mask: 1 where k <= q (upper incl diag), else 0
    tri = np.tri(128, 128, 0, dtype=np.float32).T.astype(ml_dtypes.bfloat16)
    in_maps = []
    for core in range(8):
        b = core // 4
        hg = core % 4
        c0 = hg * CLOC
        csl = slice(c0, c0 + CLOC)

        # packed consts: [128, 144] bf16-typed raw columns
        cst = np.zeros((128, 144), np.uint16)
        cst[:, 0:128] = tri.view(np.uint16)
        bq = np.ascontiguousarray(
            b_qkv[csl].astype(np.float32).reshape(2, 128).T
        )
        bk = np.ascontiguousarray(
            b_qkv[DM + c0 : DM + c0 + CLOC].astype(np.float32).reshape(2, 128).T
        )
        bv = np.ascontiguousarray(
            b_qkv[2 * DM + c0 : 2 * DM + c0 + CLOC]
            .astype(np.float32).reshape(HLOC, 64).T
        )
        cst[:, 128:132] = bq.view(np.uint16).reshape(128, 4)
        cst[:, 132:136] = bk.view(np.uint16).reshape(128, 4)
        cst[0:64, 136:144] = bv.view(np.uint16).reshape(64, 8)

        wqk = np.concatenate(
            [_pack_w(w_qkv[:, csl]), _pack_w(w_qkv[:, DM + c0 : DM + c0 + CLOC])],
            axis=2,
        )
        in_maps.append(
            {
                "xT": _pack_w(np.ascontiguousarray(x[b].T)),
                "wqk": np.ascontiguousarray(wqk),
                "wv": _pack_w(w_qkv[:, 2 * DM + c0 : 2 * DM + c0 + CLOC]),
                # wo: [CLOC, DM] -> [128, 2, DM]
                "wo": np.ascontiguousarray(
                    w_out[csl, :].reshape(2, 128, DM).transpose(1, 0, 2)
                ).astype(ml_dtypes.bfloat16),
                "cst": cst.view(ml_dtypes.bfloat16),
            }
        )
    return in_maps


def gather(results, b_out):
    out = np.empty((B, S, DM), np.float32)
    for b in range(B):
        acc = results[4 * b]["y"].astype(np.float32)
        for j in range(1, 4):
            acc = acc + results[4 * b + j]["y"]
        out[b] = acc + b_out.astype(np.float32)[None, :]
    return out


def kernel(x, w_qkv, b_qkv, w_out, b_out):
    x = np.asarray(x)
    w_qkv = np.asarray(w_qkv)
    b_qkv = np.asarray(b_qkv)
    w_out = np.asarray(w_out)
    b_out = np.asarray(b_out)

    if "nc" not in _CACHE:
        _CACHE["nc"] = build()
    nc = _CACHE["nc"]

    in_maps = make_in_maps(x, w_qkv, b_qkv, w_out)
    res = run_bass_kernel_spmd(nc, in_maps, core_ids=list(range(8)))
    return gather(res.results, b_out)


# revision 43
# speedup vs baseline: 427.9433x; 1.0097x over previous
"""Causal multi-head attention block (B=2, S=2048, D=1024, H=16) on 8 TRN2 cores.

Sharding: core i handles batch b = i//4 and head group hg = i%4 (4 heads =
256 model dims). Each core computes its heads' attention and a partial
output projection; the host sums the 4 partials per batch and adds b_out.

Per-core device pipeline (bf16 matmuls, fp32 PSUM accumulation):
  1. QKV. Q^T,K^T land as [head_cols, tokens] (lhsT = W, rhs = x^T);
     V lands as [tokens, head_cols] (lhsT = x^T tiles, rhs = W_v) and is
     stored augmented with a ones column so the attention z-matmul also
     produces softmax row sums.
  2. Attention per head, flash-style in the S^T = K.Q^T orientation over
     the causal lower triangle only: S^T[k_tile, q_span] -> exp on ScalarE
     (scale=1/8; no max subtraction, logits are ~N(0,1)) -> P^T bf16 ->
     multiplicative 0/1 mask on the diagonal block -> z^T[d+1, q] +=
     V_aug^T @ P^T accumulated over k tiles in PSUM. Consecutive k tiles
     share one S region so each exp call covers up to 1024 columns.
  3. Normalize as soon as a q-quarter's last k tile lands: recip(rowsum),
     GPSIMD partition-broadcast, z * recip + b_v on VectorE -> bf16 zT.
  4. Out-proj: y_partial[t, n] accumulated over the 256 local dims.

Program order interleaves token halves -- QKV(cols 0:1024), attention
(q 0:1024), QKV(cols 1024:2048), attention(q 1024:2048), out-proj -- so
ScalarE exp work overlaps PE QKV work.
"""

import numpy as np
import ml_dtypes

import concourse.mybir as mybir
import concourse.tile as tile
from concourse import bacc
from concourse.bass_utils import run_bass_kernel_spmd

B = 2
S = 2048
DM = 1024
HD = 64
HLOC = 4                 # heads per core
CLOC = HLOC * HD         # local model dims (256)
MO = DM // 128           # 8 k-subtiles of the model dim
NKT = S // 128           # 16 key tiles


f32 = mybir.dt.float32
bf16 = mybir.dt.bfloat16
EXP = mybir.ActivationFunctionType.Exp

_CACHE = {}


def build(ps_z_bufs=2, pt_bufs=5, op_engines=("dve", "dve", "dve", "mix"),
          interleave_heads=False):
    nc = bacc.Bacc("TRN2", target_bir_lowering=False, debug=False)

    xT_d = nc.dram_tensor("xT", [128, MO, S], bf16, kind="ExternalInput")
    wqk_d = nc.dram_tensor("wqk", [128, MO, 2 * CLOC], bf16, kind="ExternalInput")
    wv_d = nc.dram_tensor("wv", [128, MO, CLOC], bf16, kind="ExternalInput")
    wo_d = nc.dram_tensor("wo", [128, 2, DM], bf16, kind="ExternalInput")
    # consts packed as raw bf16 columns: tri[0:128], bq[128:132],
    # bk[132:136], bv[136:144] (f32 values bit-split across bf16 pairs)
    cst_d = nc.dram_tensor("cst", [128, 144], bf16, kind="ExternalInput")
    y_d = nc.dram_tensor("y", [S, DM], f32, kind="ExternalOutput")

    with tile.TileContext(nc) as tc:
        with (
            tc.tile_pool(name="consts", bufs=1) as consts,
            tc.tile_pool(name="acts", bufs=1) as apool,
            tc.tile_pool(name="pt", bufs=pt_bufs) as ppool,
            tc.tile_pool(name="norm", bufs=4) as spool,
            tc.tile_pool(name="ycopy", bufs=4) as ypool,
            # 8 PSUM banks: ps_s 2x[128,1024]=4 (QKV Q/K + attention S),
            # ps_z [128,512] z accumulators, ps_b rest (V / out-proj)
            tc.tile_pool(name="ps_s", bufs=2, space="PSUM") as ps_s,
            tc.tile_pool(name="ps_z", bufs=ps_z_bufs, space="PSUM") as ps_z,
            tc.tile_pool(name="ps_b", bufs=8 - 4 - ps_z_bufs, space="PSUM") as ps_b,
        ):
            csb = consts.tile([128, 144], bf16)
            wqk = consts.tile([128, MO, 2 * CLOC], bf16)
            wv = consts.tile([128, MO, CLOC], bf16)
            wo = consts.tile([128, 2, DM], bf16)
            xT = apool.tile([128, MO, S], bf16)

            # DMA order = consumption order. First chunks are fine-grained
            # so the first QKV matmul starts ~2.5us in; the rest are big
            # transfers to minimize per-DMA descriptor overhead.
            nc.sync.dma_start(wqk[:, 0:3, 0:256], wqk_d[:, 0:3, 0:256])
            nc.scalar.dma_start(xT[:, 0:3, 0:512], xT_d[:, 0:3, 0:512])
            nc.sync.dma_start(csb[:], cst_d[:])
            nc.sync.dma_start(wqk[:, 3:MO, 0:256], wqk_d[:, 3:MO, 0:256])
            nc.scalar.dma_start(xT[:, 3:MO, 0:512], xT_d[:, 3:MO, 0:512])
            nc.sync.dma_start(wqk[:, :, 256:512], wqk_d[:, :, 256:512])
            nc.scalar.dma_start(wv[:], wv_d[:])
            nc.sync.dma_start(xT[:, :, 512:1024], xT_d[:, :, 512:1024])
            nc.scalar.dma_start(xT[:, :, 1024:1536], xT_d[:, :, 1024:1536])
            nc.sync.dma_start(xT[:, :, 1536:2048], xT_d[:, :, 1536:2048])
            nc.scalar.dma_start(wo[:], wo_d[:])

            tri = csb[:, 0:128]
            bq_sb = csb[:, 128:132].bitcast(f32)
            bk_sb = csb[:, 132:136].bitcast(f32)
            bv_sb = csb[0:64, 136:144].bitcast(f32)

            QT = apool.tile([128, 2, S], bf16)
            KT = apool.tile([128, 2, S], bf16)
            # V augmented: [t-part, kt, h, 0:64] = v dims, col 64 = ones
            VA = apool.tile([128, NKT, HLOC, 72], bf16)
            nc.vector.memset(VA[:, :, :, 64:65], 1.0)
            zT = apool.tile([128, 2, S], bf16)

            def emit_qkv(tg):
                tsl = slice(tg * 512, (tg + 1) * 512)
                for ct in range(2):
                    for j, (dst, b_sb) in enumerate(
                        ((QT, bq_sb), (KT, bk_sb))
                    ):
                        csl = slice(ct * 256 + j * 128, ct * 256 + (j + 1) * 128)
                        ps = ps_s.tile([128, 1024], f32, tag="s")
                        for mo in range(MO):
                            nc.tensor.matmul(
                                ps[:, 0:512],
                                wqk[:, mo, csl],
                                xT[:, mo, tsl],
                                start=(mo == 0),
                                stop=(mo == MO - 1),
                            )
                        nc.vector.tensor_scalar_add(
                            dst[:, ct, tsl], ps[:, 0:512], b_sb[:, ct : ct + 1]
                        )
                for ti in range(4):
                    tt = tg * 4 + ti
                    ps = ps_b.tile([128, 512], f32, tag="b")
                    for mo in range(MO):
                        nc.tensor.matmul(
                            ps[:, 0:CLOC],
                            xT[:, mo, tt * 128 : (tt + 1) * 128],
                            wv[:, mo, :],
                            start=(mo == 0),
                            stop=(mo == MO - 1),
                        )
                    if tg == 0:
                        nc.scalar.copy(
                            VA[:, tt, :, 0:64],
                            ps[:, 0:CLOC].rearrange("p (h d) -> p h d", d=64),
                        )
                    else:
                        nc.vector.tensor_copy(
                            VA[:, tt, :, 0:64],
                            ps[:, 0:CLOC].rearrange("p (h d) -> p h d", d=64),
                        )

            def emit_attention(qg):
                g0 = qg * 512
                last_kt = 4 * qg + 3

                # pack consecutive k tiles into shared S regions so one
                # exp call covers up to 1024 columns
                groups, cur, cum = [], [], 0
                for kt in range(last_kt + 1):
                    w = g0 + 512 - max(kt * 128, g0)
                    if cum + w > 1024:
                        groups.append(cur)
                        cur, cum = [], 0
                    cur.append((kt, cum, w))
                    cum += w
                groups.append(cur)

                head_lists = ([0, 1, 3, 2] if not interleave_heads
                              else [[0, 1], [2, 3]])
                for hl in head_lists:
                    hs = [hl] if isinstance(hl, int) else hl
                    zp_map = {}
                    for h in hs:
                        zp_map[h] = ps_z.tile([128, 512], f32, tag="z",
                                              name=f"zps_{h}_{qg}")
                    for grp in groups:
                        for h in hs:
                            emit_head_grp(h, grp, zp_map[h], qg, g0, last_kt)
                    for h in hs:
                        emit_norm(h, zp_map[h], qg, g0)

            def emit_head_grp(h, grp, zp, qg, g0, last_kt):
                    hp = (h % 2) * 64
                    ct = h // 2
                    if True:
                        sreg = ps_s.tile([128, 1024], f32, tag="s",
                                         name=f"s_{h}_{qg}_{grp[0][0]}")
                        cum = grp[-1][1] + grp[-1][2]
                        for kt, off, w in grp:
                            q0 = g0 + 512 - w
                            c0 = off
                            while c0 < off + w:
                                cw = min(off + w - c0, 512 - c0 % 512)
                                nc.tensor.matmul(
                                    sreg[:, c0 : c0 + cw],
                                    KT[hp : hp + 64, ct,
                                       kt * 128 : (kt + 1) * 128],
                                    QT[hp : hp + 64, ct,
                                       q0 + c0 - off : q0 + c0 - off + cw],
                                )
                                c0 += cw
                        pT = ppool.tile([128, 1024], bf16, tag="pT")
                        nc.scalar.activation(
                            pT[:, :cum], sreg[:, :cum], EXP, scale=0.125
                        )
                        for kt, off, w in grp:
                            if kt * 128 >= g0:  # diagonal block leads span
                                nc.vector.tensor_mul(
                                    pT[:, off : off + 128],
                                    pT[:, off : off + 128],
                                    tri[:],
                                )
                            q0 = g0 + 512 - w
                            nc.tensor.matmul(
                                zp[0:65, q0 - g0 : 512],
                                VA[:, kt, h, 0:65],
                                pT[:, off : off + w],
                                start=(kt == 0),
                                stop=(kt == last_kt),
                            )

            def emit_norm(h, zp, qg, g0):
                    hp = (h % 2) * 64
                    ct = h // 2
                    rec32 = spool.tile([1, 512], f32, tag="rec32",
                                       name=f"rec_{h}_{qg}")
                    nc.vector.reciprocal(rec32[:], zp[64:65, 0:512])
                    bcast = spool.tile([64, 512], f32, tag="bcast",
                                       name=f"bc_{h}_{qg}")
                    nc.gpsimd.partition_broadcast(bcast[:], rec32[:])
                    zmul = spool.tile([64, 512], f32, tag="zmul",
                                      name=f"zm_{h}_{qg}")
                    nc.vector.tensor_mul(zmul[:], zp[0:64, 0:512], bcast[:])
                    if hp == 0:
                        nc.vector.tensor_scalar_add(
                            zT[0:64, ct, g0 : g0 + 512], zmul[:],
                            bv_sb[:, h : h + 1],
                        )
                    else:
                        zbf = spool.tile([64, 512], bf16, tag="zbf",
                                         name=f"zb_{h}_{qg}")
                        nc.vector.tensor_scalar_add(
                            zbf[:], zmul[:], bv_sb[:, h : h + 1]
                        )
                        nc.sync.dma_start(
                            zT[hp : hp + 64, ct, g0 : g0 + 512], zbf[:]
                        )

            def emit_outproj(qg, copy_eng, dma_split=1):
                for nh in range(2):
                    ysb = ypool.tile([128, 4, 512], f32, tag="y",
                                     name=f"ysb_{qg}_{nh}")
                    nper = 4 // dma_split
                    for ti in range(4):
                        tt = qg * 4 + ti
                        ps = ps_b.tile([128, 512], f32, tag="b")
                        for co in range(2):
                            nc.tensor.matmul(
                                ps[:],
                                zT[:, co, tt * 128 : (tt + 1) * 128],
                                wo[:, co, nh * 512 : (nh + 1) * 512],
                                start=(co == 0),
                                stop=(co == 1),
                            )
                        eng = copy_eng if copy_eng != "mix" else (
                            "act" if (tt + nh) % 2 == 0 else "dve"
                        )
                        if eng == "act":
                            nc.scalar.copy(ysb[:, ti, :], ps[:])
                        else:
                            nc.vector.tensor_copy(ysb[:, ti, :], ps[:])
                        if ti % nper == nper - 1:
                            t0 = tt - nper + 1
                            deng = nc.sync if (ti // nper + nh) % 2 == 0 else nc.scalar
                            deng.dma_start(
                                y_d[t0 * 128 : (tt + 1) * 128,
                                    nh * 512 : (nh + 1) * 512].rearrange(
                                    "(ti p) n -> p ti n", p=128
                                ),
                                ysb[:, ti - nper + 1 : ti + 1, :],
                            )

            # 4-stage software pipeline: attention on quarter qg overlaps
            # the QKV projection of quarter qg+1 on PE
            emit_qkv(0)
            emit_attention(0)
            emit_qkv(1)
            emit_attention(1)
            emit_qkv(2)
            emit_attention(2)
            emit_qkv(3)
            emit_attention(3)
            for qg in range(4):
                emit_outproj(qg, op_engines[qg], dma_split=2 if qg == 3 else 1)

    nc.compile()
    return nc


def _pack_w(w):
    # [DM, C] -> [128, MO, C]: partition p holds rows {mo*128 + p}
    return np.ascontiguousarray(
        w.reshape(MO, 128, w.shape[1]).transpose(1, 0, 2)
    ).astype(ml_dtypes.bfloat16)


def make_in_maps(x, w_qkv, b_qkv, w_out):
    # multiplicative post-exp mask: 1 where k <= q (upper incl diag), else 0
    tri = np.tri(128, 128, 0, dtype=np.float32).T.astype(ml_dtypes.bfloat16)
    in_maps = []
    for core in range(8):
        b = core // 4
        hg = core % 4
        c0 = hg * CLOC
        csl = slice(c0, c0 + CLOC)

        # packed consts: [128, 144] bf16-typed raw columns
        cst = np.zeros((128, 144), np.uint16)
        cst[:, 0:128] = tri.view(np.uint16)
        bq = np.ascontiguousarray(
            b_qkv[csl].astype(np.float32).reshape(2, 128).T
        )
        bk = np.ascontiguousarray(
            b_qkv[DM + c0 : DM + c0 + CLOC].astype(np.float32).reshape(2, 128).T
        )
        bv = np.ascontiguousarray(
            b_qkv[2 * DM + c0 : 2 * DM + c0 + CLOC]
            .astype(np.float32).reshape(HLOC, 64).T
        )
        cst[:, 128:132] = bq.view(np.uint16).reshape(128, 4)
        cst[:, 132:136] = bk.view(np.uint16).reshape(128, 4)
        cst[0:64, 136:144] = bv.view(np.uint16).reshape(64, 8)

        wq_p = _pack_w(w_qkv[:, csl])
        wk_p = _pack_w(w_qkv[:, DM + c0 : DM + c0 + CLOC])
        wqk = np.concatenate(
            [wq_p[:, :, 0:128], wk_p[:, :, 0:128],
             wq_p[:, :, 128:256], wk_p[:, :, 128:256]],
            axis=2,
        )
        in_maps.append(
            {
                "xT": _pack_w(np.ascontiguousarray(x[b].T)),
                "wqk": np.ascontiguousarray(wqk),
                "wv": _pack_w(w_qkv[:, 2 * DM + c0 : 2 * DM + c0 + CLOC]),
                # wo: [CLOC, DM] -> [128, 2, DM]
                "wo": np.ascontiguousarray(
                    w_out[csl, :].reshape(2, 128, DM).transpose(1, 0, 2)
                ).astype(ml_dtypes.bfloat16),
                "cst": cst.view(ml_dtypes.bfloat16),
            }
        )
    return in_maps


def gather(results, b_out):
    out = np.empty((B, S, DM), np.float32)
    for b in range(B):
        acc = results[4 * b]["y"].astype(np.float32)
        for j in range(1, 4):
            acc = acc + results[4 * b + j]["y"]
        out[b] = acc + b_out.astype(np.float32)[None, :]
    return out


def kernel(x, w_qkv, b_qkv, w_out, b_out):
    x = np.asarray(x)
    w_qkv = np.asarray(w_qkv)
    b_qkv = np.asarray(b_qkv)
    w_out = np.asarray(w_out)
    b_out = np.asarray(b_out)

    if "nc" not in _CACHE:
        _CACHE["nc"] = build()
    nc = _CACHE["nc"]

    in_maps = make_in_maps(x, w_qkv, b_qkv, w_out)
    res = run_bass_kernel_spmd(nc, in_maps, core_ids=list(range(8)))
    return gather(res.results, b_out)


# revision 48
# speedup vs baseline: 432.0650x; 1.0096x over previous
"""Causal multi-head attention block (B=2, S=2048, D=1024, H=16) on 8 TRN2 cores.

Sharding: core i handles batch b = i//4 and head group hg = i%4 (4 heads =
256 model dims). Each core computes its heads' attention and a partial
output projection; the host sums the 4 partials per batch and adds b_out.

Per-core device pipeline (bf16 matmuls, fp32 PSUM accumulation):
  1. QKV. Q^T,K^T land as [head_cols, tokens] (lhsT = W, rhs = x^T);
     V lands as [tokens, head_cols] (lhsT = x^T tiles, rhs = W_v) and is
     stored augmented with a ones column so the attention z-matmul also
     produces softmax row sums.
  2. Attention per head, flash-style in the S^T = K.Q^T orientation over
     the causal lower triangle only: S^T[k_tile, q_span] -> exp on ScalarE
     (scale=1/8; no max subtraction, logits are ~N(0,1)) -> P^T bf16 ->
     multiplicative 0/1 mask on the diagonal block -> z^T[d+1, q] +=
     V_aug^T @ P^T accumulated over k tiles in PSUM. Consecutive k tiles
     share one S region so each exp call covers up to 1024 columns.
  3. Normalize as soon as a q-quarter's last k tile lands: recip(rowsum),
     GPSIMD partition-broadcast, z * recip on VectorE -> bf16 zT. The V
     bias is folded into the output bias on the host (b_v @ w_out).
  4. Out-proj: y_partial[t, n] accumulated over the 256 local dims.

Program order is a 4-stage pipeline over 512-token quarters --
QKV(tg0), att(qg0), QKV(tg1), att(qg1), ... out-proj last -- so ScalarE
exp work overlaps PE QKV work and out-proj fills late PE gaps. Host
pre-packs all inputs into SBUF layouts (bf16) for contiguous DMA.
"""

import numpy as np
import ml_dtypes

import concourse.mybir as mybir
import concourse.tile as tile
from concourse import bacc
from concourse.bass_utils import run_bass_kernel_spmd

B = 2
S = 2048
DM = 1024
HD = 64
HLOC = 4                 # heads per core
CLOC = HLOC * HD         # local model dims (256)
MO = DM // 128           # 8 k-subtiles of the model dim
NKT = S // 128           # 16 key tiles


f32 = mybir.dt.float32
bf16 = mybir.dt.bfloat16
EXP = mybir.ActivationFunctionType.Exp

_CACHE = {}


def build(ps_z_bufs=2, pt_bufs=5, op_engines=("dve", "dve", "dve", "mix"),
          interleave_heads=False, dma_splits=(1, 1, 1, 4)):
    nc = bacc.Bacc("TRN2", target_bir_lowering=False, debug=False)

    xT_d = nc.dram_tensor("xT", [128, MO, S], bf16, kind="ExternalInput")
    wqk_d = nc.dram_tensor("wqk", [128, MO, 2 * CLOC], bf16, kind="ExternalInput")
    wv_d = nc.dram_tensor("wv", [128, MO, CLOC], bf16, kind="ExternalInput")
    wo_d = nc.dram_tensor("wo", [128, 2, DM], bf16, kind="ExternalInput")
    # consts packed as raw bf16 columns: tri[0:128], bq[128:132],
    # bk[132:136], bv[136:144] (f32 values bit-split across bf16 pairs)
    cst_d = nc.dram_tensor("cst", [128, 144], bf16, kind="ExternalInput")
    y_d = nc.dram_tensor("y", [S, DM], f32, kind="ExternalOutput")

    with tile.TileContext(nc) as tc:
        with (
            tc.tile_pool(name="consts", bufs=1) as consts,
            tc.tile_pool(name="acts", bufs=1) as apool,
            tc.tile_pool(name="pt", bufs=pt_bufs) as ppool,
            tc.tile_pool(name="norm", bufs=4) as spool,
            tc.tile_pool(name="ycopy", bufs=4) as ypool,
            # 8 PSUM banks: ps_s 2x[128,1024]=4 (QKV Q/K + attention S),
            # ps_z [128,512] z accumulators, ps_b rest (V / out-proj)
            tc.tile_pool(name="ps_s", bufs=2, space="PSUM") as ps_s,
            tc.tile_pool(name="ps_z", bufs=ps_z_bufs, space="PSUM") as ps_z,
            tc.tile_pool(name="ps_b", bufs=8 - 4 - ps_z_bufs, space="PSUM") as ps_b,
        ):
            csb = consts.tile([128, 144], bf16)
            wqk = consts.tile([128, MO, 2 * CLOC], bf16)
            wv = consts.tile([128, MO, CLOC], bf16)
            wo = consts.tile([128, 2, DM], bf16)
            xT = apool.tile([128, MO, S], bf16)

            # DMA order = consumption order. First chunks are fine-grained
            # so the first QKV matmul starts ~2.5us in; the rest are big
            # transfers to minimize per-DMA descriptor overhead.
            nc.sync.dma_start(wqk[:, 0:3, 0:256], wqk_d[:, 0:3, 0:256])
            nc.scalar.dma_start(xT[:, 0:3, 0:512], xT_d[:, 0:3, 0:512])
            nc.sync.dma_start(csb[:], cst_d[:])
            nc.sync.dma_start(wqk[:, 3:MO, 0:256], wqk_d[:, 3:MO, 0:256])
            nc.scalar.dma_start(xT[:, 3:MO, 0:512], xT_d[:, 3:MO, 0:512])
            nc.sync.dma_start(wqk[:, :, 256:512], wqk_d[:, :, 256:512])
            nc.scalar.dma_start(wv[:], wv_d[:])
            nc.sync.dma_start(xT[:, :, 512:1024], xT_d[:, :, 512:1024])
            nc.scalar.dma_start(xT[:, :, 1024:1536], xT_d[:, :, 1024:1536])
            nc.sync.dma_start(xT[:, :, 1536:2048], xT_d[:, :, 1536:2048])
            nc.scalar.dma_start(wo[:], wo_d[:])

            tri = csb[:, 0:128]
            bq_sb = csb[:, 128:132].bitcast(f32)
            bk_sb = csb[:, 132:136].bitcast(f32)

            QT = apool.tile([128, 2, S], bf16)
            KT = apool.tile([128, 2, S], bf16)
            # V augmented: [t-part, kt, h, 0:64] = v dims, col 64 = ones
            VA = apool.tile([128, NKT, HLOC, 72], bf16)
            nc.vector.memset(VA[:, :, :, 64:65], 1.0)
            zT = apool.tile([128, 2, S], bf16)

            def emit_qkv(tg):
                tsl = slice(tg * 512, (tg + 1) * 512)
                for ct in range(2):
                    for j, (dst, b_sb) in enumerate(
                        ((QT, bq_sb), (KT, bk_sb))
                    ):
                        csl = slice(ct * 256 + j * 128, ct * 256 + (j + 1) * 128)
                        ps = ps_s.tile([128, 1024], f32, tag="s")
                        for mo in range(MO):
                            nc.tensor.matmul(
                                ps[:, 0:512],
                                wqk[:, mo, csl],
                                xT[:, mo, tsl],
                                start=(mo == 0),
                                stop=(mo == MO - 1),
                            )
                        nc.vector.tensor_scalar_add(
                            dst[:, ct, tsl], ps[:, 0:512], b_sb[:, ct : ct + 1]
                        )
                for ti in range(4):
                    tt = tg * 4 + ti
                    ps = ps_b.tile([128, 512], f32, tag="b")
                    for mo in range(MO):
                        nc.tensor.matmul(
                            ps[:, 0:CLOC],
                            xT[:, mo, tt * 128 : (tt + 1) * 128],
                            wv[:, mo, :],
                            start=(mo == 0),
                            stop=(mo == MO - 1),
                        )
                    if tg == 0:
                        nc.scalar.copy(
                            VA[:, tt, :, 0:64],
                            ps[:, 0:CLOC].rearrange("p (h d) -> p h d", d=64),
                        )
                    else:
                        nc.vector.tensor_copy(
                            VA[:, tt, :, 0:64],
                            ps[:, 0:CLOC].rearrange("p (h d) -> p h d", d=64),
                        )

            def emit_attention(qg):
                g0 = qg * 512
                last_kt = 4 * qg + 3

                # pack consecutive k tiles into shared S regions so one
                # exp call covers up to 1024 columns
                groups, cur, cum = [], [], 0
                for kt in range(last_kt + 1):
                    w = g0 + 512 - max(kt * 128, g0)
                    if cum + w > 1024:
                        groups.append(cur)
                        cur, cum = [], 0
                    cur.append((kt, cum, w))
                    cum += w
                groups.append(cur)

                head_lists = ([0, 1, 3, 2] if not interleave_heads
                              else [[0, 1], [2, 3]])
                for hl in head_lists:
                    hs = [hl] if isinstance(hl, int) else hl
                    zp_map = {}
                    for h in hs:
                        zp_map[h] = ps_z.tile([128, 512], f32, tag="z",
                                              name=f"zps_{h}_{qg}")
                    for grp in groups:
                        for h in hs:
                            emit_head_grp(h, grp, zp_map[h], qg, g0, last_kt)
                    for h in hs:
                        emit_norm(h, zp_map[h], qg, g0)

            def emit_head_grp(h, grp, zp, qg, g0, last_kt):
                    hp = (h % 2) * 64
                    ct = h // 2
                    if True:
                        sreg = ps_s.tile([128, 1024], f32, tag="s",
                                         name=f"s_{h}_{qg}_{grp[0][0]}")
                        cum = grp[-1][1] + grp[-1][2]
                        for kt, off, w in grp:
                            q0 = g0 + 512 - w
                            c0 = off
                            while c0 < off + w:
                                cw = min(off + w - c0, 512 - c0 % 512)
                                nc.tensor.matmul(
                                    sreg[:, c0 : c0 + cw],
                                    KT[hp : hp + 64, ct,
                                       kt * 128 : (kt + 1) * 128],
                                    QT[hp : hp + 64, ct,
                                       q0 + c0 - off : q0 + c0 - off + cw],
                                )
                                c0 += cw
                        pT = ppool.tile([128, 1024], bf16, tag="pT")
                        nc.scalar.activation(
                            pT[:, :cum], sreg[:, :cum], EXP, scale=0.125
                        )
                        for kt, off, w in grp:
                            if kt * 128 >= g0:  # diagonal block leads span
                                nc.vector.tensor_mul(
                                    pT[:, off : off + 128],
                                    pT[:, off : off + 128],
                                    tri[:],
                                )
                            q0 = g0 + 512 - w
                            nc.tensor.matmul(
                                zp[0:65, q0 - g0 : 512],
                                VA[:, kt, h, 0:65],
                                pT[:, off : off + w],
                                start=(kt == 0),
                                stop=(kt == last_kt),
                            )

            def emit_norm(h, zp, qg, g0):
                    hp = (h % 2) * 64
                    ct = h // 2
                    rec32 = spool.tile([1, 512], f32, tag="rec32",
                                       name=f"rec_{h}_{qg}")
                    nc.vector.reciprocal(rec32[:], zp[64:65, 0:512])
                    bcast = spool.tile([64, 512], f32, tag="bcast",
                                       name=f"bc_{h}_{qg}")
                    nc.gpsimd.partition_broadcast(bcast[:], rec32[:])
                    # b_v is folded into b_out on the host:
                    # y += (1 (x) b_v) @ w_out is a constant row vector
                    with nc.allow_low_precision(reason="attn out to bf16"):
                        if hp == 0:
                            nc.vector.tensor_mul(
                                zT[0:64, ct, g0 : g0 + 512],
                                zp[0:64, 0:512], bcast[:],
                            )
                        else:
                            zbf = spool.tile([64, 512], bf16, tag="zbf",
                                             name=f"zb_{h}_{qg}")
                            nc.vector.tensor_mul(
                                zbf[:], zp[0:64, 0:512], bcast[:]
                            )
                            nc.sync.dma_start(
                                zT[hp : hp + 64, ct, g0 : g0 + 512], zbf[:]
                            )

            def emit_outproj(qg, copy_eng, dma_split=1):
                for nh in range(2):
                    ysb = ypool.tile([128, 4, 512], f32, tag="y",
                                     name=f"ysb_{qg}_{nh}")
                    nper = 4 // dma_split
                    for ti in range(4):
                        tt = qg * 4 + ti
                        ps = ps_b.tile([128, 512], f32, tag="b")
                        for co in range(2):
                            nc.tensor.matmul(
                                ps[:],
                                zT[:, co, tt * 128 : (tt + 1) * 128],
                                wo[:, co, nh * 512 : (nh + 1) * 512],
                                start=(co == 0),
                                stop=(co == 1),
                            )
                        eng = copy_eng if copy_eng != "mix" else (
                            "act" if (tt + nh) % 2 == 0 else "dve"
                        )
                        if eng == "act":
                            nc.scalar.copy(ysb[:, ti, :], ps[:])
                        else:
                            nc.vector.tensor_copy(ysb[:, ti, :], ps[:])
                        if ti % nper == nper - 1:
                            t0 = tt - nper + 1
                            deng = nc.sync if (ti // nper + nh) % 2 == 0 else nc.scalar
                            deng.dma_start(
                                y_d[t0 * 128 : (tt + 1) * 128,
                                    nh * 512 : (nh + 1) * 512].rearrange(
                                    "(ti p) n -> p ti n", p=128
                                ),
                                ysb[:, ti - nper + 1 : ti + 1, :],
                            )

            # 4-stage software pipeline: attention on quarter qg overlaps
            # the QKV projection of quarter qg+1 on PE
            emit_qkv(0)
            emit_attention(0)
            emit_qkv(1)
            emit_attention(1)
            emit_qkv(2)
            emit_attention(2)
            emit_qkv(3)
            emit_attention(3)
            for qg in range(4):
                emit_outproj(qg, op_engines[qg], dma_split=dma_splits[qg])

    nc.compile()
    return nc


def _pack_w(w):
    # [DM, C] -> [128, MO, C]: partition p holds rows {mo*128 + p}
    return np.ascontiguousarray(
        w.reshape(MO, 128, w.shape[1]).transpose(1, 0, 2)
    ).astype(ml_dtypes.bfloat16)


def make_in_maps(x, w_qkv, b_qkv, w_out):
    # multiplicative post-exp mask: 1 where k <= q (upper incl diag), else 0
    tri = np.tri(128, 128, 0, dtype=np.float32).T.astype(ml_dtypes.bfloat16)
    in_maps = []
    for core in range(8):
        b = core // 4
        hg = core % 4
        c0 = hg * CLOC
        csl = slice(c0, c0 + CLOC)

        # packed consts: [128, 144] bf16-typed raw columns
        cst = np.zeros((128, 144), np.uint16)
        cst[:, 0:128] = tri.view(np.uint16)
        bq = np.ascontiguousarray(
            b_qkv[csl].astype(np.float32).reshape(2, 128).T
        )
        bk = np.ascontiguousarray(
            b_qkv[DM + c0 : DM + c0 + CLOC].astype(np.float32).reshape(2, 128).T
        )
        bv = np.ascontiguousarray(
            b_qkv[2 * DM + c0 : 2 * DM + c0 + CLOC]
            .astype(np.float32).reshape(HLOC, 64).T
        )
        cst[:, 128:132] = bq.view(np.uint16).reshape(128, 4)
        cst[:, 132:136] = bk.view(np.uint16).reshape(128, 4)
        cst[0:64, 136:144] = bv.view(np.uint16).reshape(64, 8)

        wq_p = _pack_w(w_qkv[:, csl])
        wk_p = _pack_w(w_qkv[:, DM + c0 : DM + c0 + CLOC])
        wqk = np.concatenate(
            [wq_p[:, :, 0:128], wk_p[:, :, 0:128],
             wq_p[:, :, 128:256], wk_p[:, :, 128:256]],
            axis=2,
        )
        in_maps.append(
            {
                "xT": _pack_w(np.ascontiguousarray(x[b].T)),
                "wqk": np.ascontiguousarray(wqk),
                "wv": _pack_w(w_qkv[:, 2 * DM + c0 : 2 * DM + c0 + CLOC]),
                # wo: [CLOC, DM] -> [128, 2, DM]
                "wo": np.ascontiguousarray(
                    w_out[csl, :].reshape(2, 128, DM).transpose(1, 0, 2)
                ).astype(ml_dtypes.bfloat16),
                "cst": cst.view(ml_dtypes.bfloat16),
            }
        )
    return in_maps


def gather(results, b_qkv, w_out, b_out):
    # device skips the V bias; z_norm + b_v projects to a constant row:
    # y += b_v @ w_out, folded into the output bias here
    b_eff = (
        b_out.astype(np.float32)
        + b_qkv[2 * DM :].astype(np.float32) @ w_out.astype(np.float32)
    )
    out = np.empty((B, S, DM), np.float32)
    for b in range(B):
        acc = results[4 * b]["y"].astype(np.float32)
        for j in range(1, 4):
            acc = acc + results[4 * b + j]["y"]
        out[b] = acc + b_eff[None, :]
    return out


def kernel(x, w_qkv, b_qkv, w_out, b_out):
    x = np.asarray(x)
    w_qkv = np.asarray(w_qkv)
    b_qkv = np.asarray(b_qkv)
    w_out = np.asarray(w_out)
    b_out = np.asarray(b_out)

    if "nc" not in _CACHE:
        _CACHE["nc"] = build()
    nc = _CACHE["nc"]

    in_maps = make_in_maps(x, w_qkv, b_qkv, w_out)
    res = run_bass_kernel_spmd(nc, in_maps, core_ids=list(range(8)))
    return gather(res.results, b_qkv, w_out, b_out)
